# revision 13
# baseline (speedup 1.0000x reference)
"""Trainium2 Bass kernel: decoder GQA attention with RoPE, tensor-parallel over 8 NeuronCores.

Sharding: core c = (h, g) with h = c//4, g = c%4 handles the 4 query heads of
GQA group g (heads 4g..4g+3, which share KV head g) for the 2 batches
{2h, 2h+1}.  Compared with 2-heads x 4-batches per core this removes the
duplicated K/V projections entirely (48 instead of 64 projection matmuls per
token chunk) without any extra collective, halves the x DMA, and widens the
output-projection matmuls to 512 moving columns.  All matmul operands are
bf16 (same PE rate as fp32r, half the DMA/SBUF traffic); PSUM and softmax
denominators stay fp32.  Per core:
  - Constants (RoPE tables, causal masks, Wo^T, bias) are DMA'd at kernel
    start so the attention phase never waits on them; Wo^T chunks are
    interleaved with the projection units to stay off the x-stream's critical
    path.
  - QKV projection of the core's 2 batches against its [C, 768] weight slice,
    emitted output-major (q01 | q23 | kv accumulation groups) so each PSUM
    ring slot drains while the next group computes; RoPE on the fly; q/k/v
    stay SBUF-resident.
  - Flash-style causal attention with transposed scores (sT[k,q]) in
    [128,1024] PSUM tiles, exp batched per 1024 cols on the Scalar engine,
    software-pipelined so PV matmuls of the previous key-chunk fill the PE
    while the current chunk exponentiates.  Scores AND PV matmuls in the
    fully-masked region of diagonal tiles are skipped (partial moving dims;
    the PV split carries per-column-segment stop flags).  The softmax
    denominator is a bf16 fold-tree on the Vector engine plus one ones-matmul
    per query chunk; normalization uses the fast approximate reciprocal.
  - One 8-core AllToAll per local batch index reshards the attention output
    head->token in 256-token chunks (cores 0-3 contribute batch lb, cores
    4-7 batch 2+lb, so every core ends with both batches' full channels for
    its token slice and the output projection keeps 512 moving columns);
    wo(lb=0) is emission-interleaved with attn(lb=1) so its matmuls fill the
    exp-pipeline bubbles and hide the collective; bias is fused into the
    Scalar PSUM->SBUF copy; the host transposes at assembly.
"""

import os
import sys

for _p in ("/opt/trn_rl_repo",):
    if _p not in sys.path:
        sys.path.insert(0, _p)

import numpy as np
from ml_dtypes import bfloat16

import concourse.bacc as bacc
import concourse.mybir as mybir
import concourse.tile as tile
from concourse.bass_utils import run_bass_kernel_spmd

F32 = mybir.dt.float32
BF16 = mybir.dt.bfloat16
AX = mybir.AluOpType

B, T, C = 4, 2048, 2048
N_HEAD, N_KV = 16, 4
HD = C // N_HEAD            # 128
NCORES = 8
GROUPS = [list(range(NCORES))]
HPC = 4                     # q heads per core (one full GQA group)
BL = 2                      # local batches per core
SCALE = 1.0 / float(np.sqrt(HD))
TQ = 512                    # query-chunk (psum free dim)
NQC = T // TQ               # 4 query chunks per (lb, head)
CCH = C // 128              # 16 contraction chunks
TW = T // NCORES            # 256 tokens per (core, batch) in the wo shard

_CACHE = {}


def _build():
    """Build + compile the per-core Bass graph (same graph for every core)."""
    nc = bacc.Bacc(
        "TRN2",
        target_bir_lowering=False,
        debug=False,
        enable_asserts=False,
        num_devices=NCORES,
    )

    xt_d = nc.dram_tensor("xt", [BL, C, T], BF16, kind="ExternalInput")
    wqkv_d = nc.dram_tensor("wqkv", [C, 768], BF16, kind="ExternalInput")
    wot_d = nc.dram_tensor("wot", [C, C], BF16, kind="ExternalInput")
    cc_d = nc.dram_tensor("ropec", [128, T], BF16, kind="ExternalInput")
    ss_d = nc.dram_tensor("ropes", [128, T], BF16, kind="ExternalInput")
    mask_d = nc.dram_tensor("masks", [128, 4 * TQ], F32, kind="ExternalInput")
    ones_d = nc.dram_tensor("ones", [128, 128], BF16, kind="ExternalInput")
    ident_d = nc.dram_tensor("ident", [128, 128], BF16, kind="ExternalInput")
    boc_d = nc.dram_tensor("boc", [128, CCH], F32, kind="ExternalInput")
    out_d = nc.dram_tensor("out", [C, BL * 2 * TW], F32, kind="ExternalOutput")

    with tile.TileContext(nc) as tc:
        with tc.tile_pool(name="dram", bufs=1, space="DRAM") as dp:
            in_bufs = [
                dp.tile([2 * C, TW], BF16, name=f"in_buf{b}") for b in range(BL)
            ]
            out_bufs = [
                dp.tile([2 * C, TW], BF16, name=f"out_buf{b}") for b in range(BL)
            ]
            warm_in = dp.tile([NCORES, 64], BF16, name="warm_in")
            warm_out = dp.tile([NCORES, 64], BF16, name="warm_out")
            nc.gpsimd.collective_compute(
                "AllToAll",
                AX.bypass,
                replica_groups=GROUPS,
                ins=[warm_in.opt()],
                outs=[warm_out.opt()],
            )
            with tc.tile_pool(name="res", bufs=1) as rp:
                # constants resident for the whole kernel, loaded up front
                id_sb = rp.tile([128, 128], BF16, name="id_sb")
                nc.sync.dma_start(out=id_sb[:], in_=ident_d.ap())
                cc_sb = rp.tile([128, T], BF16, name="cc_sb")
                nc.sync.dma_start(out=cc_sb[:], in_=cc_d.ap())
                ss_sb = rp.tile([128, T], BF16, name="ss_sb")
                nc.sync.dma_start(out=ss_sb[:], in_=ss_d.ap())
                mask_sb = rp.tile([128, 4 * TQ], F32, name="mask_sb")
                nc.sync.dma_start(out=mask_sb[:], in_=mask_d.ap())
                ones_sb = rp.tile([128, 128], BF16, name="ones_sb")
                nc.sync.dma_start(out=ones_sb[:], in_=ones_d.ap())
                boc_sb = rp.tile([128, CCH], F32, name="boc_sb")
                nc.sync.dma_start(out=boc_sb[:], in_=boc_d.ap())
                wot_sb = rp.tile([128, CCH * C], BF16, name="wot_sb")

                kt_all = rp.tile([128, BL * T], BF16, name="kt_all")
                vstd_all = rp.tile([128, BL * T], BF16, name="vstd_all")
                q_all = rp.tile([128, HPC * BL * T], BF16, name="q_all")

                _phase1_qkv(nc, tc, xt_d, wqkv_d, wot_d, id_sb, cc_sb, ss_sb,
                            wot_sb, q_all, kt_all, vstd_all)
                _phase2_attn_wo(nc, tc, mask_sb, ones_sb, boc_sb, wot_sb,
                                q_all, kt_all, vstd_all, in_bufs, out_bufs,
                                out_d)

    nc.compile()
    return nc


def _phase1_qkv(nc, tc, xt_d, wqkv_d, wot_d, id_sb, cc_sb, ss_sb, wot_sb,
                q_all, kt_all, vstd_all):
    with (
        tc.tile_pool(name="p1c", bufs=1) as p1c,
        tc.tile_pool(name="px", bufs=32) as px,
        tc.tile_pool(name="pt", bufs=3) as pt,
        tc.tile_pool(name="pp", bufs=3, space="PSUM") as pp,
        tc.tile_pool(name="pst", bufs=2, space="PSUM") as pst,
    ):
        # q01 weight columns + first x unit first, so the first accumulation
        # group's matmuls start ~3MB of DMA earlier than a full-weight load
        w_sb = p1c.tile([128, CCH * 768], BF16, name="w_sb")
        warm_xts = []
        for ci in range(CCH):
            nc.sync.dma_start(
                out=w_sb[:, ci * 768 : ci * 768 + 256],
                in_=wqkv_d[ci * 128 : (ci + 1) * 128, 0:256],
            )
            xtile = px.tile([128, TQ], BF16, tag="xt", name="xt")
            nc.sync.dma_start(
                out=xtile[:], in_=xt_d[0, ci * 128 : (ci + 1) * 128, 0:TQ]
            )
            warm_xts.append(xtile)
        for ci in range(CCH):
            nc.sync.dma_start(
                out=w_sb[:, ci * 768 + 256 : (ci + 1) * 768],
                in_=wqkv_d[ci * 128 : (ci + 1) * 128, 256:768],
            )

        def rope(psrc, dst_ap, cs):
            # dst = src*cc + swap_halves(src)*ss   (rotate-half RoPE)
            qs = pt.tile([128, TQ], BF16, tag="qs", name="qs")
            nc.scalar.copy(qs[:], psrc)
            qsw = pt.tile([128, TQ], BF16, tag="qsw", name="qsw")
            nc.sync.dma_start(out=qsw[0:64, :], in_=qs[64:128, :])
            nc.sync.dma_start(out=qsw[64:128, :], in_=qs[0:64, :])
            tm1 = pt.tile([128, TQ], BF16, tag="tm1", name="tm1")
            nc.vector.tensor_tensor(tm1[:], qs[:], cc_sb[:, cs], AX.mult)
            tm2 = pt.tile([128, TQ], BF16, tag="tm2", name="tm2")
            nc.vector.tensor_tensor(tm2[:], qsw[:], ss_sb[:, cs], AX.mult)
            nc.vector.tensor_tensor(dst_ap, tm1[:], tm2[:], AX.add)

        unit = 0
        for lb in range(BL):
            for n in range(NQC):
                if lb == 0 and n == 0:
                    xts = warm_xts
                else:
                    xts = []
                    for ci in range(CCH):
                        xtile = px.tile([128, TQ], BF16, tag="xt", name="xt")
                        nc.sync.dma_start(
                            out=xtile[:],
                            in_=xt_d[
                                lb, ci * 128 : (ci + 1) * 128, n * TQ : (n + 1) * TQ
                            ],
                        )
                        xts.append(xtile)
                cs = slice(n * TQ, (n + 1) * TQ)
                # output-major accumulation groups: q01 | q23 | kv, so each
                # PSUM ring slot is drained while the next group computes
                ps = [
                    pp.tile([128, 2 * TQ], F32, tag="proj", name=f"ps{i}")
                    for i in range(3)
                ]
                for grp in range(3):
                    for ci in range(CCH):
                        for half in range(2):
                            m = 2 * grp + half
                            nc.tensor.matmul(
                                ps[grp][:, half * TQ : (half + 1) * TQ],
                                w_sb[:, ci * 768 + m * 128 : ci * 768 + (m + 1) * 128],
                                xts[ci][:],
                                start=(ci == 0),
                                stop=(ci == CCH - 1),
                            )
                    if grp < 2:
                        for half in range(2):
                            hl = 2 * grp + half
                            rope(ps[grp][:, half * TQ : (half + 1) * TQ],
                                 q_all[:, (hl * BL + lb) * T + n * TQ :
                                       (hl * BL + lb) * T + (n + 1) * TQ], cs)
                    else:
                        rope(ps[grp][:, 0:TQ],
                             kt_all[:, lb * T + n * TQ : lb * T + (n + 1) * TQ],
                             cs)
                        vt = pt.tile([128, TQ], BF16, tag="vt", name="vt")
                        nc.scalar.copy(vt[:], ps[grp][:, TQ : 2 * TQ])
                        ptr = pst.tile([128, TQ], BF16, tag="vtr", name="vtr")
                        for i in range(TQ // 128):
                            nc.tensor.transpose(
                                ptr[:, i * 128 : (i + 1) * 128],
                                vt[:, i * 128 : (i + 1) * 128],
                                id_sb[:],
                            )
                        nc.scalar.copy(
                            vstd_all[:, lb * T + n * TQ : lb * T + (n + 1) * TQ],
                            ptr[:],
                        )
                # stream Wo^T in behind this unit's x tiles (2 chunks/unit)
                for jc in (2 * unit, 2 * unit + 1):
                    nc.sync.dma_start(
                        out=wot_sb[:, jc * C : (jc + 1) * C],
                        in_=wot_d[jc * 128 : (jc + 1) * 128, :],
                    )
                unit += 1


def _phase2_attn_wo(nc, tc, mask_sb, ones_sb, boc_sb, wot_sb, q_all, kt_all,
                    vstd_all, in_bufs, out_bufs, out_d):
    with (
        tc.tile_pool(name="pe", bufs=9) as pe,
        tc.tile_pool(name="pd", bufs=3) as pd,
        tc.tile_pool(name="pn", bufs=4) as pn,
        tc.tile_pool(name="pr", bufs=3) as pr,
        tc.tile_pool(name="pa", bufs=20) as pa,
        tc.tile_pool(name="po", bufs=4) as po,
        tc.tile_pool(name="pss", bufs=3, space="PSUM") as pss,
        tc.tile_pool(name="pso", bufs=1, space="PSUM") as pso,
    ):
        def attn_unit(lb, hl, qcg):
            qb = (hl * BL + lb) * T
            qcs = (2 * qcg, 2 * qcg + 1)
            kimax = [qc * 4 + 3 for qc in qcs]
            notrim = lb == 0 and hl == 0 and qcg == 0
            q_aps = [
                q_all[:, qb + qc * TQ : qb + (qc + 1) * TQ] for qc in qcs
            ]
            psum_o = [
                pso.tile([128, TQ], F32, tag=f"o{qi}", name=f"po{qi}")
                for qi in range(2)
            ]
            accs = [None, None]

            def emit_s(k0):
                exps = {}
                for kp in range(2):
                    klo = k0 + 2 * kp
                    for qi, qc in enumerate(qcs):
                        if klo > kimax[qi]:
                            continue
                        ps_s = pss.tile([128, 2 * TQ], F32, tag="s", name="pss")
                        for j in range(2):
                            ki = klo + j
                            di = ki - qc * 4
                            lo = di * 128 if (di > 0 and not notrim) else 0
                            nc.tensor.matmul(
                                ps_s[:, j * TQ + lo : (j + 1) * TQ],
                                kt_all[:, lb * T + ki * 128 :
                                       lb * T + (ki + 1) * 128],
                                q_aps[qi][:, lo:TQ],
                                start=True,
                                stop=True,
                            )
                        for j in range(2):
                            ki = klo + j
                            di = ki - qc * 4
                            if di >= 0:
                                w = (di + 1) * 128
                                nc.vector.tensor_tensor(
                                    ps_s[:, j * TQ : j * TQ + w],
                                    ps_s[:, j * TQ : j * TQ + w],
                                    mask_sb[:, di * TQ : di * TQ + w],
                                    AX.add,
                                )
                        ex_sb = pe.tile([128, 2 * TQ], BF16, tag="e", name="ex")
                        nc.scalar.activation(
                            ex_sb[:],
                            ps_s[:],
                            mybir.ActivationFunctionType.Exp,
                            scale=SCALE,
                        )
                        exps[(qi, kp)] = ex_sb
                return exps

            def emit_pvd(k0, exps):
                for kp in range(2):
                    for j in range(2):
                        ki = k0 + 2 * kp + j
                        vsl = vstd_all[
                            :, lb * T + ki * 128 : lb * T + (ki + 1) * 128
                        ]
                        for qi in range(2):
                            if ki > kimax[qi] or (qi, kp) not in exps:
                                continue
                            nc.tensor.matmul(
                                psum_o[qi][:],
                                vsl,
                                exps[(qi, kp)][:, j * TQ : (j + 1) * TQ],
                                start=(ki == 0),
                                stop=(ki == kimax[qi]),
                            )
                # denominator fold tree: per-tile folds on the otherwise-idle
                # GpSimd engine (Vector is the attention-phase pacing engine),
                # pair/acc adds stay on Vector
                for qi in range(2):
                    folds = []
                    for kp in range(2):
                        if (qi, kp) not in exps:
                            continue
                        ex_sb = exps[(qi, kp)]
                        f = pd.tile([128, TQ], BF16, tag="f", name="f")
                        nc.gpsimd.tensor_tensor(
                            f[:], ex_sb[:, 0:TQ], ex_sb[:, TQ : 2 * TQ],
                            AX.add,
                        )
                        folds.append(f)
                    if not folds:
                        continue
                    if len(folds) == 2:
                        cs_t = pd.tile([128, TQ], BF16, tag="cs", name="cs")
                        nc.vector.tensor_tensor(
                            cs_t[:], folds[0][:], folds[1][:], AX.add
                        )
                    else:
                        cs_t = folds[0]
                    if accs[qi] is None:
                        accs[qi] = cs_t
                    else:
                        na = pd.tile([128, TQ], BF16, tag=f"a{qi}", name="acc")
                        nc.vector.tensor_tensor(
                            na[:], accs[qi][:], cs_t[:], AX.add
                        )
                        accs[qi] = na

            pending = None
            for k0 in range(0, kimax[1] + 1, 4):
                exps = emit_s(k0)
                if pending is not None:
                    emit_pvd(*pending)
                pending = (k0, exps)
            emit_pvd(*pending)

            ps_df = pss.tile([128, 2 * TQ], F32, tag="s", name="pdf")
            for qi in range(2):
                nc.tensor.matmul(
                    ps_df[:, qi * TQ : (qi + 1) * TQ],
                    ones_sb[:], accs[qi][:],
                    start=True, stop=True,
                )
            for qi, qc in enumerate(qcs):
                rec = pr.tile([128, TQ], F32, tag="r", name="rec")
                nc.vector.reciprocal_approx_fast(
                    rec[:], ps_df[:, qi * TQ : (qi + 1) * TQ]
                )
                onrm = pn.tile([128, TQ], BF16, tag="on", name="onrm")
                nc.vector.tensor_tensor(
                    onrm[:], psum_o[qi][:], rec[:], AX.mult
                )
                for half in range(2):
                    j = 2 * qc + half
                    nc.sync.dma_start(
                        out=in_bufs[lb][
                            j * 512 + hl * 128 : j * 512 + (hl + 1) * 128, :
                        ],
                        in_=onrm[:, half * TW : (half + 1) * TW],
                    )

        def emit_a2a(lb):
            nc.gpsimd.collective_compute(
                "AllToAll",
                AX.bypass,
                replica_groups=GROUPS,
                ins=[in_bufs[lb].opt()],
                outs=[out_bufs[lb].opt()],
            )

        def load_atts(lb):
            # cols 0:256 = batch lb, cols 256:512 = batch 2+lb (same tokens)
            atts = []
            for jc in range(CCH):
                a = pa.tile([128, 2 * TW], BF16, tag="att", name="att")
                nc.sync.dma_start(
                    out=a[:, 0:TW],
                    in_=out_bufs[lb][jc * 128 : (jc + 1) * 128, :],
                )
                nc.sync.dma_start(
                    out=a[:, TW : 2 * TW],
                    in_=out_bufs[lb][C + jc * 128 : C + (jc + 1) * 128, :],
                )
                atts.append(a)
            return atts

        def wo_unit(lb, atts, cs):
            psum = pss.tile([128, 2 * TQ], F32, tag="s", name="pwo")
            for jc in range(CCH):
                nc.tensor.matmul(
                    psum[:, 0 : 2 * TW],
                    wot_sb[:, jc * C + cs * 128 : jc * C + (cs + 1) * 128],
                    atts[jc][:],
                    start=(jc == 0),
                    stop=(jc == CCH - 1),
                )
            osb = po.tile([128, 2 * TW], F32, tag="ou", name="osb")
            nc.scalar.activation(
                osb[:],
                psum[:, 0 : 2 * TW],
                mybir.ActivationFunctionType.Identity,
                bias=boc_sb[:, cs : cs + 1],
            )
            nc.sync.dma_start(
                out=out_d[
                    cs * 128 : (cs + 1) * 128, lb * 2 * TW : (lb + 1) * 2 * TW
                ],
                in_=osb[:],
            )

        # Sequential emission: the scheduler fills late attn(1) stalls with
        # wo(0) matmuls once the (over-estimated) modeled A2A completes, and
        # the full attn(1) batch of PE work separates A2A(0) from the first
        # wo(0) matmul in the in-order PE queue, so the collective is always
        # hidden.  (Interleaving wo units into attn emission puts
        # A2A-dependent matmuls too early in the PE queue and stalls it.)
        for hl in range(HPC):
            for qcg in range(2):
                attn_unit(0, hl, qcg)
        emit_a2a(0)
        for hl in range(HPC):
            for qcg in range(2):
                attn_unit(1, hl, qcg)
        atts0 = load_atts(0)
        for cs in range(CCH):
            wo_unit(0, atts0, cs)
        emit_a2a(1)
        atts1 = load_atts(1)
        for cs in range(CCH):
            wo_unit(1, atts1, cs)


def _prep_inputs(x, rope_cos, rope_sin, Wq, Wkv, Wo, bo):
    x = np.asarray(x, np.float32)
    rope_cos = np.asarray(rope_cos, np.float32)
    rope_sin = np.asarray(rope_sin, np.float32)
    Wq = np.asarray(Wq, np.float32)
    Wkv = np.asarray(Wkv, np.float32)
    Wo = np.asarray(Wo, np.float32)
    bo = np.asarray(bo, np.float32)

    xt = np.ascontiguousarray(x.transpose(0, 2, 1)).astype(bfloat16)  # (B, C, T)
    wot = np.ascontiguousarray(Wo.T).astype(bfloat16)                 # (j, c_out)
    cc = np.concatenate([rope_cos.T, rope_cos.T], axis=0).astype(bfloat16)
    ss = np.concatenate([-rope_sin.T, rope_sin.T], axis=0).astype(bfloat16)

    masks = np.zeros((128, 4 * TQ), np.float32)
    kp = np.arange(128)[:, None]
    qf = np.arange(TQ)[None, :]
    for di in range(4):
        masks[:, di * TQ : (di + 1) * TQ] = np.where(kp + di * 128 <= qf, 0.0, -1e30)

    ones = np.ones((128, 128), bfloat16)
    ident = np.eye(128, dtype=np.float32).astype(bfloat16)
    boc = np.ascontiguousarray(bo.reshape(CCH, 128).T)  # [p, cs]

    in_maps = []
    for c in range(NCORES):
        h, g = c // 4, c % 4
        wqkv = np.ascontiguousarray(
            np.concatenate(
                [Wq[(4 * g + m) * HD : (4 * g + m + 1) * HD, :].T
                 for m in range(4)]
                + [
                    Wkv[g * HD : (g + 1) * HD, :].T,
                    Wkv[N_KV * HD + g * HD : N_KV * HD + (g + 1) * HD, :].T,
                ],
                axis=1,
            )
        ).astype(bfloat16)
        in_maps.append(
            {
                "xt": np.ascontiguousarray(xt[2 * h : 2 * h + 2]),
                "wqkv": wqkv,
                "wot": wot,
                "ropec": cc,
                "ropes": ss,
                "masks": masks,
                "ones": ones,
                "ident": ident,
                "boc": boc,
            }
        )
    return in_maps


def kernel(x, rope_cos, rope_sin, Wq, Wkv, Wo, bo):
    if "nc" not in _CACHE:
        _CACHE["nc"] = _build()
    nc = _CACHE["nc"]
    in_maps = _prep_inputs(x, rope_cos, rope_sin, Wq, Wkv, Wo, bo)

    trace = bool(int(os.environ.get("KERNEL_TRACE", "0")))
    kw = {}
    if trace:
        _install_trace_hook()
        kw["trace"] = True
    res = run_bass_kernel_spmd(nc, in_maps, core_ids=list(range(NCORES)), **kw)
    _CACHE["exec_time_ns"] = res.exec_time_ns

    # per-core out is [C, BL*2*TW]: token slice [c*TW:(c+1)*TW] of batches
    # (lb, 2+lb) packed per lb; reassemble
    out = np.empty((B, T, C), np.float32)
    for c in range(NCORES):
        o = res.results[c]["out"]  # (C, 1024)
        for lb in range(BL):
            out[lb, c * TW : (c + 1) * TW, :] = o[
                :, lb * 2 * TW : lb * 2 * TW + TW
            ].T
            out[2 + lb, c * TW : (c + 1) * TW, :] = o[
                :, lb * 2 * TW + TW : (lb + 1) * 2 * TW
            ].T
    return out


def _install_trace_hook():
    """Register the NTFF profiling hook (missing antenv.axon_hooks shim)."""
    import types

    import antenv
    from concourse import bass_utils

    if not hasattr(antenv, "axon_hooks"):
        mod = types.ModuleType("antenv.axon_hooks")
        hook = [None]
        mod.set_axon_ntff_profile_hook = lambda h: hook.__setitem__(0, h)
        mod.get_axon_ntff_profile_hook = lambda: hook[0]
        sys.modules["antenv.axon_hooks"] = mod
        antenv.axon_hooks = mod
        try:
            from trn_agent_boot.trn_boot import _ntff_profile_via_ctypes

            mod.set_axon_ntff_profile_hook(
                _ntff_profile_via_ctypes("/opt/axon/libaxon_pjrt.so")
            )
        except Exception:
            pass
    bass_utils.upload_artifacts = lambda tmpdir: f"local://{tmpdir}"


# revision 19
# speedup vs baseline: 1.0244x; 1.0244x over previous
"""Trainium2 Bass kernel: decoder GQA attention with RoPE, tensor-parallel over 8 NeuronCores.

Sharding: core c = (h, g) with h = c//4, g = c%4 handles the 4 query heads of
GQA group g (heads 4g..4g+3, which share KV head g) for the 2 batches
{2h, 2h+1}.  Compared with 2-heads x 4-batches per core this removes the
duplicated K/V projections entirely (48 instead of 64 projection matmuls per
token chunk) without any extra collective, halves the x DMA, and widens the
output-projection matmuls to 512 moving columns.  All matmul operands are
bf16 (same PE rate as fp32r, half the DMA/SBUF traffic); PSUM and softmax
denominators stay fp32.  Per core:
  - Constants (RoPE tables, causal masks, Wo^T, bias) are DMA'd at kernel
    start so the attention phase never waits on them; Wo^T chunks are
    interleaved with the projection units to stay off the x-stream's critical
    path.
  - QKV projection of the core's 2 batches against its [C, 768] weight slice,
    emitted output-major (q01 | q23 | kv accumulation groups) so each PSUM
    ring slot drains while the next group computes; RoPE on the fly; q/k/v
    stay SBUF-resident.
  - Flash-style causal attention with transposed scores (sT[k,q]) in
    [128,1024] PSUM tiles, exp batched per 1024 cols on the Scalar engine,
    software-pipelined so PV matmuls of the previous key-chunk fill the PE
    while the current chunk exponentiates.  Scores AND PV matmuls in the
    fully-masked region of diagonal tiles are skipped (partial moving dims;
    the PV split carries per-column-segment stop flags).  The softmax
    denominator is a bf16 fold-tree on the Vector engine plus one ones-matmul
    per query chunk; normalization uses the fast approximate reciprocal.
  - One 8-core AllToAll per local batch index reshards the attention output
    head->token in 256-token chunks (cores 0-3 contribute batch lb, cores
    4-7 batch 2+lb, so every core ends with both batches' full channels for
    its token slice and the output projection keeps 512 moving columns);
    wo(lb=0) is emission-interleaved with attn(lb=1) so its matmuls fill the
    exp-pipeline bubbles and hide the collective; bias is fused into the
    Scalar PSUM->SBUF copy; the host transposes at assembly.
"""

import os
import sys

for _p in ("/opt/trn_rl_repo",):
    if _p not in sys.path:
        sys.path.insert(0, _p)

import numpy as np
from ml_dtypes import bfloat16

import concourse.bacc as bacc
import concourse.mybir as mybir
import concourse.tile as tile
from concourse.bass_utils import run_bass_kernel_spmd

F32 = mybir.dt.float32
BF16 = mybir.dt.bfloat16
AX = mybir.AluOpType

B, T, C = 4, 2048, 2048
N_HEAD, N_KV = 16, 4
HD = C // N_HEAD            # 128
NCORES = 8
GROUPS = [list(range(NCORES))]
HPC = 4                     # q heads per core (one full GQA group)
BL = 2                      # local batches per core
SCALE = 1.0 / float(np.sqrt(HD))
TQ = 512                    # query-chunk (psum free dim)
NQC = T // TQ               # 4 query chunks per (lb, head)
CCH = C // 128              # 16 contraction chunks
TW = T // NCORES            # 256 tokens per (core, batch) in the wo shard

_CACHE = {}


def _build():
    """Build + compile the per-core Bass graph (same graph for every core)."""
    nc = bacc.Bacc(
        "TRN2",
        target_bir_lowering=False,
        debug=False,
        enable_asserts=False,
        num_devices=NCORES,
    )

    xt_d = nc.dram_tensor("xt", [BL, C, T], BF16, kind="ExternalInput")
    wqkv_d = nc.dram_tensor("wqkv", [C, 768], BF16, kind="ExternalInput")
    wot_d = nc.dram_tensor("wot", [C, C], BF16, kind="ExternalInput")
    cc_d = nc.dram_tensor("ropec", [128, T], BF16, kind="ExternalInput")
    ss_d = nc.dram_tensor("ropes", [128, T], BF16, kind="ExternalInput")
    mask_d = nc.dram_tensor("masks", [128, 4 * TQ], F32, kind="ExternalInput")
    ones_d = nc.dram_tensor("ones", [128, 128], BF16, kind="ExternalInput")
    ident_d = nc.dram_tensor("ident", [128, 128], BF16, kind="ExternalInput")
    boc_d = nc.dram_tensor("boc", [128, CCH], F32, kind="ExternalInput")
    out_d = nc.dram_tensor("out", [C, BL * 2 * TW], F32, kind="ExternalOutput")

    with tile.TileContext(nc) as tc:
        with tc.tile_pool(name="dram", bufs=1, space="DRAM") as dp:
            in_bufs = [
                dp.tile([2 * C, TW], BF16, name=f"in_buf{b}") for b in range(BL)
            ]
            out_bufs = [
                dp.tile([2 * C, TW], BF16, name=f"out_buf{b}") for b in range(BL)
            ]
            warm_in = dp.tile([NCORES, 64], BF16, name="warm_in")
            warm_out = dp.tile([NCORES, 64], BF16, name="warm_out")
            nc.gpsimd.collective_compute(
                "AllToAll",
                AX.bypass,
                replica_groups=GROUPS,
                ins=[warm_in.opt()],
                outs=[warm_out.opt()],
            )
            with tc.tile_pool(name="res", bufs=1) as rp:
                # constants resident for the whole kernel; DMAs are emitted by
                # phase 1 after the first unit's w/x loads so the first
                # matmuls are not queued behind them
                id_sb = rp.tile([128, 128], BF16, name="id_sb")
                cc_sb = rp.tile([128, T], BF16, name="cc_sb")
                ss_sb = rp.tile([128, T], BF16, name="ss_sb")
                mask_sb = rp.tile([128, 4 * TQ], F32, name="mask_sb")
                ones_sb = rp.tile([128, 128], BF16, name="ones_sb")
                boc_sb = rp.tile([128, CCH], F32, name="boc_sb")
                wot_sb = rp.tile([128, CCH * C], BF16, name="wot_sb")

                def emit_consts():
                    nc.sync.dma_start(out=id_sb[:], in_=ident_d.ap())
                    nc.sync.dma_start(out=cc_sb[:], in_=cc_d.ap())
                    nc.sync.dma_start(out=ss_sb[:], in_=ss_d.ap())
                    nc.sync.dma_start(out=mask_sb[:], in_=mask_d.ap())
                    nc.sync.dma_start(out=ones_sb[:], in_=ones_d.ap())
                    nc.sync.dma_start(out=boc_sb[:], in_=boc_d.ap())

                kt_all = rp.tile([128, BL * T], BF16, name="kt_all")
                vstd_all = rp.tile([128, BL * T], BF16, name="vstd_all")
                q_all = rp.tile([128, HPC * BL * T], BF16, name="q_all")

                _phase1_qkv(nc, tc, xt_d, wqkv_d, wot_d, id_sb, cc_sb, ss_sb,
                            wot_sb, q_all, kt_all, vstd_all, emit_consts)
                _phase2_attn_wo(nc, tc, mask_sb, ones_sb, boc_sb, wot_sb,
                                q_all, kt_all, vstd_all, in_bufs, out_bufs,
                                out_d)

    nc.compile()
    return nc


def _phase1_qkv(nc, tc, xt_d, wqkv_d, wot_d, id_sb, cc_sb, ss_sb, wot_sb,
                q_all, kt_all, vstd_all, emit_consts):
    with (
        tc.tile_pool(name="p1c", bufs=1) as p1c,
        tc.tile_pool(name="px", bufs=32) as px,
        tc.tile_pool(name="pt", bufs=3) as pt,
        tc.tile_pool(name="pp", bufs=3, space="PSUM") as pp,
        tc.tile_pool(name="pst", bufs=2, space="PSUM") as pst,
    ):
        # first unit's weights (Sync queue) and x tiles (GpSimd queue) issue
        # in parallel so the first ci-major matmul group starts after ~2 DMAs
        w_sb = p1c.tile([128, CCH * 768], BF16, name="w_sb")
        warm_xts = []
        for ci in range(CCH):
            nc.sync.dma_start(
                out=w_sb[:, ci * 768 : (ci + 1) * 768],
                in_=wqkv_d[ci * 128 : (ci + 1) * 128, :],
            )
            xtile = px.tile([128, TQ], BF16, tag="xt", name="xt")
            nc.gpsimd.dma_start(
                out=xtile[:], in_=xt_d[0, ci * 128 : (ci + 1) * 128, 0:TQ]
            )
            warm_xts.append(xtile)
        emit_consts()

        def rope(psrc, dst_ap, cs):
            # dst = src*cc + swap_halves(src)*ss   (rotate-half RoPE)
            qs = pt.tile([128, TQ], BF16, tag="qs", name="qs")
            nc.scalar.copy(qs[:], psrc)
            qsw = pt.tile([128, TQ], BF16, tag="qsw", name="qsw")
            nc.scalar.dma_start(out=qsw[0:64, :], in_=qs[64:128, :])
            nc.scalar.dma_start(out=qsw[64:128, :], in_=qs[0:64, :])
            tm1 = pt.tile([128, TQ], BF16, tag="tm1", name="tm1")
            nc.vector.tensor_tensor(tm1[:], qs[:], cc_sb[:, cs], AX.mult)
            tm2 = pt.tile([128, TQ], BF16, tag="tm2", name="tm2")
            nc.vector.tensor_tensor(tm2[:], qsw[:], ss_sb[:, cs], AX.mult)
            nc.vector.tensor_tensor(dst_ap, tm1[:], tm2[:], AX.add)

        unit = 0
        for lb in range(BL):
            for n in range(NQC):
                if lb == 0 and n == 0:
                    xts = warm_xts
                else:
                    xts = []
                    for ci in range(CCH):
                        xtile = px.tile([128, TQ], BF16, tag="xt", name="xt")
                        nc.gpsimd.dma_start(
                            out=xtile[:],
                            in_=xt_d[
                                lb, ci * 128 : (ci + 1) * 128, n * TQ : (n + 1) * TQ
                            ],
                        )
                        xts.append(xtile)
                cs = slice(n * TQ, (n + 1) * TQ)
                # output-major accumulation groups: q01 | q23 | kv, so each
                # PSUM ring slot is drained while the next group computes.
                # The first unit is ci-major instead: its matmuls then only
                # wait for w[0]/x[0] rather than the full weight+x load.
                ps = [
                    pp.tile([128, 2 * TQ], F32, tag="proj", name=f"ps{i}")
                    for i in range(3)
                ]
                if unit == 0:
                    for ci in range(CCH):
                        for m in range(6):
                            nc.tensor.matmul(
                                ps[m // 2][:, (m % 2) * TQ : (m % 2 + 1) * TQ],
                                w_sb[:, ci * 768 + m * 128 : ci * 768 + (m + 1) * 128],
                                xts[ci][:],
                                start=(ci == 0),
                                stop=(ci == CCH - 1),
                            )
                for grp in range(3):
                    if unit > 0:
                        for ci in range(CCH):
                            for half in range(2):
                                m = 2 * grp + half
                                nc.tensor.matmul(
                                    ps[grp][:, half * TQ : (half + 1) * TQ],
                                    w_sb[:, ci * 768 + m * 128 : ci * 768 + (m + 1) * 128],
                                    xts[ci][:],
                                    start=(ci == 0),
                                    stop=(ci == CCH - 1),
                                )
                    if grp < 2:
                        for half in range(2):
                            hl = 2 * grp + half
                            rope(ps[grp][:, half * TQ : (half + 1) * TQ],
                                 q_all[:, (hl * BL + lb) * T + n * TQ :
                                       (hl * BL + lb) * T + (n + 1) * TQ], cs)
                    else:
                        rope(ps[grp][:, 0:TQ],
                             kt_all[:, lb * T + n * TQ : lb * T + (n + 1) * TQ],
                             cs)
                        vt = pt.tile([128, TQ], BF16, tag="vt", name="vt")
                        nc.scalar.copy(vt[:], ps[grp][:, TQ : 2 * TQ])
                        ptr = pst.tile([128, TQ], BF16, tag="vtr", name="vtr")
                        for i in range(TQ // 128):
                            nc.tensor.transpose(
                                ptr[:, i * 128 : (i + 1) * 128],
                                vt[:, i * 128 : (i + 1) * 128],
                                id_sb[:],
                            )
                        nc.scalar.copy(
                            vstd_all[:, lb * T + n * TQ : lb * T + (n + 1) * TQ],
                            ptr[:],
                        )
                # stream Wo^T in behind this unit's x tiles (2 chunks/unit)
                for jc in (2 * unit, 2 * unit + 1):
                    nc.sync.dma_start(
                        out=wot_sb[:, jc * C : (jc + 1) * C],
                        in_=wot_d[jc * 128 : (jc + 1) * 128, :],
                    )
                unit += 1


def _phase2_attn_wo(nc, tc, mask_sb, ones_sb, boc_sb, wot_sb, q_all, kt_all,
                    vstd_all, in_bufs, out_bufs, out_d):
    with (
        tc.tile_pool(name="pe", bufs=9) as pe,
        tc.tile_pool(name="pd", bufs=3) as pd,
        tc.tile_pool(name="pn", bufs=4) as pn,
        tc.tile_pool(name="pr", bufs=3) as pr,
        tc.tile_pool(name="pa", bufs=20) as pa,
        tc.tile_pool(name="po", bufs=4) as po,
        tc.tile_pool(name="pss", bufs=3, space="PSUM") as pss,
        tc.tile_pool(name="pso", bufs=1, space="PSUM") as pso,
    ):
        fold_rr = [0]

        def attn_unit(lb, hl, qcg):
            qb = (hl * BL + lb) * T
            qcs = (2 * qcg, 2 * qcg + 1)
            kimax = [qc * 4 + 3 for qc in qcs]
            notrim = lb == 0 and hl == 0 and qcg == 0
            q_aps = [
                q_all[:, qb + qc * TQ : qb + (qc + 1) * TQ] for qc in qcs
            ]
            psum_o = [
                pso.tile([128, TQ], F32, tag=f"o{qi}", name=f"po{qi}")
                for qi in range(2)
            ]
            accs = [None, None]

            def emit_s(k0):
                exps = {}
                for kp in range(2):
                    klo = k0 + 2 * kp
                    for qi, qc in enumerate(qcs):
                        if klo > kimax[qi]:
                            continue
                        ps_s = pss.tile([128, 2 * TQ], F32, tag="s", name="pss")
                        for j in range(2):
                            ki = klo + j
                            di = ki - qc * 4
                            lo = di * 128 if (di > 0 and not notrim) else 0
                            nc.tensor.matmul(
                                ps_s[:, j * TQ + lo : (j + 1) * TQ],
                                kt_all[:, lb * T + ki * 128 :
                                       lb * T + (ki + 1) * 128],
                                q_aps[qi][:, lo:TQ],
                                start=True,
                                stop=True,
                            )
                        for j in range(2):
                            ki = klo + j
                            di = ki - qc * 4
                            if di >= 0:
                                w = (di + 1) * 128
                                nc.vector.tensor_tensor(
                                    ps_s[:, j * TQ : j * TQ + w],
                                    ps_s[:, j * TQ : j * TQ + w],
                                    mask_sb[:, di * TQ : di * TQ + w],
                                    AX.add,
                                )
                        ex_sb = pe.tile([128, 2 * TQ], BF16, tag="e", name="ex")
                        nc.scalar.activation(
                            ex_sb[:],
                            ps_s[:],
                            mybir.ActivationFunctionType.Exp,
                            scale=SCALE,
                        )
                        exps[(qi, kp)] = ex_sb
                return exps

            def emit_pvd(k0, exps):
                for kp in range(2):
                    for j in range(2):
                        ki = k0 + 2 * kp + j
                        vsl = vstd_all[
                            :, lb * T + ki * 128 : lb * T + (ki + 1) * 128
                        ]
                        for qi in range(2):
                            if ki > kimax[qi] or (qi, kp) not in exps:
                                continue
                            nc.tensor.matmul(
                                psum_o[qi][:],
                                vsl,
                                exps[(qi, kp)][:, j * TQ : (j + 1) * TQ],
                                start=(ki == 0),
                                stop=(ki == kimax[qi]),
                            )
                # denominator fold tree: Vector is the attention pacing
                # engine, so route 2 of 3 per-tile folds to the idle GpSimd
                # (which costs ~2.7x per op but runs in parallel); pair/acc
                # adds stay on Vector
                for qi in range(2):
                    folds = []
                    for kp in range(2):
                        if (qi, kp) not in exps:
                            continue
                        ex_sb = exps[(qi, kp)]
                        f = pd.tile([128, TQ], BF16, tag="f", name="f")
                        eng = nc.vector if fold_rr[0] % 3 == 2 else nc.gpsimd
                        fold_rr[0] += 1
                        eng.tensor_tensor(
                            f[:], ex_sb[:, 0:TQ], ex_sb[:, TQ : 2 * TQ],
                            AX.add,
                        )
                        folds.append(f)
                    if not folds:
                        continue
                    if len(folds) == 2:
                        cs_t = pd.tile([128, TQ], BF16, tag="cs", name="cs")
                        nc.vector.tensor_tensor(
                            cs_t[:], folds[0][:], folds[1][:], AX.add
                        )
                    else:
                        cs_t = folds[0]
                    if accs[qi] is None:
                        accs[qi] = cs_t
                    else:
                        na = pd.tile([128, TQ], BF16, tag=f"a{qi}", name="acc")
                        nc.vector.tensor_tensor(
                            na[:], accs[qi][:], cs_t[:], AX.add
                        )
                        accs[qi] = na

            pending = None
            for k0 in range(0, kimax[1] + 1, 4):
                exps = emit_s(k0)
                if pending is not None:
                    emit_pvd(*pending)
                pending = (k0, exps)
            emit_pvd(*pending)

            ps_df = pss.tile([128, 2 * TQ], F32, tag="s", name="pdf")
            for qi in range(2):
                nc.tensor.matmul(
                    ps_df[:, qi * TQ : (qi + 1) * TQ],
                    ones_sb[:], accs[qi][:],
                    start=True, stop=True,
                )
            for qi, qc in enumerate(qcs):
                rec = pr.tile([128, TQ], F32, tag="r", name="rec")
                nc.vector.reciprocal_approx_fast(
                    rec[:], ps_df[:, qi * TQ : (qi + 1) * TQ]
                )
                onrm = pn.tile([128, TQ], BF16, tag="on", name="onrm")
                nc.vector.tensor_tensor(
                    onrm[:], psum_o[qi][:], rec[:], AX.mult
                )
                for half in range(2):
                    j = 2 * qc + half
                    nc.sync.dma_start(
                        out=in_bufs[lb][
                            j * 512 + hl * 128 : j * 512 + (hl + 1) * 128, :
                        ],
                        in_=onrm[:, half * TW : (half + 1) * TW],
                    )

        def emit_a2a(lb):
            nc.gpsimd.collective_compute(
                "AllToAll",
                AX.bypass,
                replica_groups=GROUPS,
                ins=[in_bufs[lb].opt()],
                outs=[out_bufs[lb].opt()],
            )

        def load_atts(lb):
            # cols 0:256 = batch lb, cols 256:512 = batch 2+lb (same tokens)
            atts = []
            for jc in range(CCH):
                a = pa.tile([128, 2 * TW], BF16, tag="att", name="att")
                nc.sync.dma_start(
                    out=a[:, 0:TW],
                    in_=out_bufs[lb][jc * 128 : (jc + 1) * 128, :],
                )
                nc.sync.dma_start(
                    out=a[:, TW : 2 * TW],
                    in_=out_bufs[lb][C + jc * 128 : C + (jc + 1) * 128, :],
                )
                atts.append(a)
            return atts

        def wo_unit(lb, atts, cs):
            psum = pss.tile([128, 2 * TQ], F32, tag="s", name="pwo")
            for jc in range(CCH):
                nc.tensor.matmul(
                    psum[:, 0 : 2 * TW],
                    wot_sb[:, jc * C + cs * 128 : jc * C + (cs + 1) * 128],
                    atts[jc][:],
                    start=(jc == 0),
                    stop=(jc == CCH - 1),
                )
            osb = po.tile([128, 2 * TW], F32, tag="ou", name="osb")
            nc.scalar.activation(
                osb[:],
                psum[:, 0 : 2 * TW],
                mybir.ActivationFunctionType.Identity,
                bias=boc_sb[:, cs : cs + 1],
            )
            nc.sync.dma_start(
                out=out_d[
                    cs * 128 : (cs + 1) * 128, lb * 2 * TW : (lb + 1) * 2 * TW
                ],
                in_=osb[:],
            )

        # Sequential emission: the scheduler fills late attn(1) stalls with
        # wo(0) matmuls once the (over-estimated) modeled A2A completes, and
        # the full attn(1) batch of PE work separates A2A(0) from the first
        # wo(0) matmul in the in-order PE queue, so the collective is always
        # hidden.  (Interleaving wo units into attn emission puts
        # A2A-dependent matmuls too early in the PE queue and stalls it.)
        for hl in range(HPC):
            for qcg in range(2):
                attn_unit(0, hl, qcg)
        emit_a2a(0)
        for hl in range(HPC):
            for qcg in range(2):
                attn_unit(1, hl, qcg)
        atts0 = load_atts(0)
        for cs in range(CCH):
            wo_unit(0, atts0, cs)
        emit_a2a(1)
        atts1 = load_atts(1)
        for cs in range(CCH):
            wo_unit(1, atts1, cs)


def _prep_inputs(x, rope_cos, rope_sin, Wq, Wkv, Wo, bo):
    x = np.asarray(x, np.float32)
    rope_cos = np.asarray(rope_cos, np.float32)
    rope_sin = np.asarray(rope_sin, np.float32)
    Wq = np.asarray(Wq, np.float32)
    Wkv = np.asarray(Wkv, np.float32)
    Wo = np.asarray(Wo, np.float32)
    bo = np.asarray(bo, np.float32)

    xt = np.ascontiguousarray(x.transpose(0, 2, 1)).astype(bfloat16)  # (B, C, T)
    wot = np.ascontiguousarray(Wo.T).astype(bfloat16)                 # (j, c_out)
    cc = np.concatenate([rope_cos.T, rope_cos.T], axis=0).astype(bfloat16)
    ss = np.concatenate([-rope_sin.T, rope_sin.T], axis=0).astype(bfloat16)

    masks = np.zeros((128, 4 * TQ), np.float32)
    kp = np.arange(128)[:, None]
    qf = np.arange(TQ)[None, :]
    for di in range(4):
        masks[:, di * TQ : (di + 1) * TQ] = np.where(kp + di * 128 <= qf, 0.0, -1e30)

    ones = np.ones((128, 128), bfloat16)
    ident = np.eye(128, dtype=np.float32).astype(bfloat16)
    boc = np.ascontiguousarray(bo.reshape(CCH, 128).T)  # [p, cs]

    in_maps = []
    for c in range(NCORES):
        h, g = c // 4, c % 4
        wqkv = np.ascontiguousarray(
            np.concatenate(
                [Wq[(4 * g + m) * HD : (4 * g + m + 1) * HD, :].T
                 for m in range(4)]
                + [
                    Wkv[g * HD : (g + 1) * HD, :].T,
                    Wkv[N_KV * HD + g * HD : N_KV * HD + (g + 1) * HD, :].T,
                ],
                axis=1,
            )
        ).astype(bfloat16)
        in_maps.append(
            {
                "xt": np.ascontiguousarray(xt[2 * h : 2 * h + 2]),
                "wqkv": wqkv,
                "wot": wot,
                "ropec": cc,
                "ropes": ss,
                "masks": masks,
                "ones": ones,
                "ident": ident,
                "boc": boc,
            }
        )
    return in_maps


def kernel(x, rope_cos, rope_sin, Wq, Wkv, Wo, bo):
    if "nc" not in _CACHE:
        _CACHE["nc"] = _build()
    nc = _CACHE["nc"]
    in_maps = _prep_inputs(x, rope_cos, rope_sin, Wq, Wkv, Wo, bo)

    trace = bool(int(os.environ.get("KERNEL_TRACE", "0")))
    kw = {}
    if trace:
        _install_trace_hook()
        kw["trace"] = True
    res = run_bass_kernel_spmd(nc, in_maps, core_ids=list(range(NCORES)), **kw)
    _CACHE["exec_time_ns"] = res.exec_time_ns

    # per-core out is [C, BL*2*TW]: token slice [c*TW:(c+1)*TW] of batches
    # (lb, 2+lb) packed per lb; reassemble
    out = np.empty((B, T, C), np.float32)
    for c in range(NCORES):
        o = res.results[c]["out"]  # (C, 1024)
        for lb in range(BL):
            out[lb, c * TW : (c + 1) * TW, :] = o[
                :, lb * 2 * TW : lb * 2 * TW + TW
            ].T
            out[2 + lb, c * TW : (c + 1) * TW, :] = o[
                :, lb * 2 * TW + TW : (lb + 1) * 2 * TW
            ].T
    return out


def _install_trace_hook():
    """Register the NTFF profiling hook (missing antenv.axon_hooks shim)."""
    import types

    import antenv
    from concourse import bass_utils

    if not hasattr(antenv, "axon_hooks"):
        mod = types.ModuleType("antenv.axon_hooks")
        hook = [None]
        mod.set_axon_ntff_profile_hook = lambda h: hook.__setitem__(0, h)
        mod.get_axon_ntff_profile_hook = lambda: hook[0]
        sys.modules["antenv.axon_hooks"] = mod
        antenv.axon_hooks = mod
        try:
            from trn_agent_boot.trn_boot import _ntff_profile_via_ctypes

            mod.set_axon_ntff_profile_hook(
                _ntff_profile_via_ctypes("/opt/axon/libaxon_pjrt.so")
            )
        except Exception:
            pass
    bass_utils.upload_artifacts = lambda tmpdir: f"local://{tmpdir}"
